# revision 11
# baseline (speedup 1.0000x reference)
"""Llama4 MoE layer on 8 Trainium2 NeuronCores — expert-parallel sparse dispatch.

Sharding strategy (the "all-to-all dispatch of top-1 routed tokens" from the
hint, done at the input-sharding step): the router is evaluated while sharding
the inputs, tokens are gathered per top-1 expert, and core c receives

  - the tokens routed to expert c (pre-scaled by sigmoid(max logit)), padded
    to C1 columns, plus
  - an even 1/8 slice of all tokens (unscaled) for the shared expert,

so each core runs ~C1+256 token-MLPs instead of the dense-masked 2048+256.
Expert outputs live on disjoint token sets and the shared slices tile the
token axis, so combining is a scatter-add — no collective needed.

Device kernel per core (identical SPMD program):
  x [P, ND*CT] bf16 -> gate/up (weight tiles stationary as lhsT, tokens
  stream) -> silu*up in f32 PSUM -> h [F, C] bf16 -> down-proj in flipped
  orientation (down tiles stationary, h streams) -> y [P, ND*CT] bf16,
  written back in one DMA at the end.
All matmuls use a full 128x128 stationary operand, so PE time ~= FLOPs/peak.
DMA-efficiency choices: gate|up fused per f-tile (16KB partition rows),
down tiles fused in pairs (8KB rows), x loaded in 4 wide-row DMAs, output
accumulated in SBUF and stored once (bf16).
"""

import sys

sys.path.insert(0, "/opt/trn_rl_repo")

import ml_dtypes
import numpy as np

import concourse.tile as tile
from concourse import bacc, mybir

T, D, F, E = 2048, 2048, 2048, 8
N_CORES = 8
P = 128
ND, NF = D // P, F // P
C2 = T // N_CORES  # shared-expert tokens per core
f32 = mybir.dt.float32
bf16 = mybir.dt.bfloat16


def build(C1):
    CT = C1 + C2
    nc = bacc.Bacc(None, target_bir_lowering=False, debug=False)
    xa = nc.declare_dram_parameter("xa", [P, ND * CT], bf16, isOutput=False)
    wgu = nc.declare_dram_parameter("wgu", [NF, P, 2 * ND * P], bf16, isOutput=False)
    wdp = nc.declare_dram_parameter(
        "wdp", [ND // 2, P, 2 * NF * P], bf16, isOutput=False
    )
    sgu = nc.declare_dram_parameter("sgu", [NF, P, 2 * ND * P], bf16, isOutput=False)
    sdp = nc.declare_dram_parameter(
        "sdp", [ND // 2, P, 2 * NF * P], bf16, isOutput=False
    )
    ye = nc.declare_dram_parameter("ye", [P, ND * C1], bf16, isOutput=True)
    ys = nc.declare_dram_parameter("ys", [P, ND * C2], bf16, isOutput=True)

    with tile.TileContext(nc) as tc:
        with (
            tc.tile_pool(name="xpool", bufs=1) as xp,
            tc.tile_pool(name="wstream", bufs=3) as wp,
            tc.tile_pool(name="hpool", bufs=2) as hp,
            tc.tile_pool(name="work", bufs=2) as sp,
            tc.tile_pool(name="psGU", bufs=2, space="PSUM") as ppG,
            tc.tile_pool(name="psD", bufs=2, space="PSUM") as ppD,
        ):
            xa_t = xp.tile([P, ND * CT], bf16, tag="xa", name="xa_t")
            nsplit = 4
            step = (ND // nsplit) * CT
            for s in range(nsplit):
                nc.sync.dma_start(
                    out=xa_t[:, step * s : step * (s + 1)],
                    in_=xa[:, step * s : step * (s + 1)],
                )
            xt = [xa_t[:, CT * d : CT * (d + 1)] for d in range(ND)]
            ye_t = xp.tile([P, ND * C1], bf16, tag="ye", name="ye_t")
            ys_t = xp.tile([P, ND * C2], bf16, tag="ys", name="ys_t")

            for w, (gu_p, dp_p, c0, C, y_t, y_p, CW) in enumerate(
                [(wgu, wdp, 0, C1, ye_t, ye, C1), (sgu, sdp, C1, C2, ys_t, ys, C2)]
            ):
                # token chunks of <=512 (PSUM bank width in f32)
                chunks = []
                q0 = 0
                while q0 < C:
                    qw = min(512, C - q0)
                    chunks.append((q0, qw))
                    q0 += qw
                # ---- gate/up -> h[f] [P, C] bf16 ----
                h_tiles = []
                for f in range(NF):
                    gu = wp.tile([P, 2 * ND * P], bf16, tag="wgu", name=f"gu{w}_{f}")
                    if w == 0 and f == 0:
                        # split the very first weight DMA so the PE can start
                        # as soon as the gate half + first x split land
                        nc.sync.dma_start(out=gu[:, : ND * P], in_=gu_p[0, :, : ND * P])
                        nc.sync.dma_start(out=gu[:, ND * P :], in_=gu_p[0, :, ND * P :])
                    else:
                        nc.sync.dma_start(out=gu[:], in_=gu_p[f])
                    gt = gu[:, : ND * P]
                    ut = gu[:, ND * P :]
                    h_t = hp.tile([P, C], bf16, tag=f"h{f}", name=f"h{w}_{f}")
                    for q0, qw in chunks:
                        pg = ppG.tile([P, qw], f32, space="PSUM", tag="pg", name="pg")
                        pu = ppG.tile([P, qw], f32, space="PSUM", tag="pu", name="pu")
                        for d in range(ND):
                            nc.tensor.matmul(
                                out=pg[:],
                                lhsT=gt[:, P * d : P * (d + 1)],
                                rhs=xt[d][:, c0 + q0 : c0 + q0 + qw],
                                start=(d == 0),
                                stop=(d == ND - 1),
                            )
                        for d in range(ND):
                            nc.tensor.matmul(
                                out=pu[:],
                                lhsT=ut[:, P * d : P * (d + 1)],
                                rhs=xt[d][:, c0 + q0 : c0 + q0 + qw],
                                start=(d == 0),
                                stop=(d == ND - 1),
                            )
                        sig = sp.tile([P, qw], f32, tag="sig", name="sig")
                        nc.scalar.activation(
                            sig[:], pg[:], mybir.ActivationFunctionType.Sigmoid
                        )
                        nc.vector.tensor_tensor(
                            out=sig[:], in0=sig[:], in1=pg[:], op=mybir.AluOpType.mult
                        )
                        nc.vector.tensor_tensor(
                            out=h_t[:, q0 : q0 + qw],
                            in0=sig[:],
                            in1=pu[:],
                            op=mybir.AluOpType.mult,
                        )
                    h_tiles.append(h_t)
                # ---- down-proj, flipped: down tiles stationary, h streams ----
                for j in range(ND // 2):
                    dd = wp.tile([P, 2 * NF * P], bf16, tag="wd", name=f"dd{w}_{j}")
                    nc.sync.dma_start(out=dd[:], in_=dp_p[j])
                    for half in range(2):
                        dblk = 2 * j + half
                        dt_ = dd[:, NF * P * half : NF * P * (half + 1)]
                        for q0, qw in chunks:
                            py = ppD.tile(
                                [P, qw], f32, space="PSUM", tag="py", name="py"
                            )
                            for f in range(NF):
                                nc.tensor.matmul(
                                    out=py[:],
                                    lhsT=dt_[:, P * f : P * (f + 1)],
                                    rhs=h_tiles[f][:, q0 : q0 + qw],
                                    start=(f == 0),
                                    stop=(f == NF - 1),
                                )
                            nc.vector.tensor_copy(
                                y_t[:, CW * dblk + q0 : CW * dblk + q0 + qw],
                                py[:],
                            )
                    # stream finished output strips out while compute continues
                    if j % 2 == 1:
                        nc.sync.dma_start(
                            out=y_p[:, CW * 2 * (j - 1) : CW * 2 * (j + 1)],
                            in_=y_t[:, CW * 2 * (j - 1) : CW * 2 * (j + 1)],
                        )
    nc.finalize()
    return nc


def _tile_lhsT(w):
    # [A, B] f32 -> [B/P, P, A] bf16 : block b, partition p(a%P), col a_blk*P+q
    A, B = w.shape
    return np.ascontiguousarray(
        w.reshape(A // P, P, B // P, P).transpose(2, 1, 0, 3).reshape(B // P, P, A)
    ).astype(ml_dtypes.bfloat16)


def _fuse_gu(g, u):
    return np.ascontiguousarray(
        np.concatenate([_tile_lhsT(g), _tile_lhsT(u)], axis=2)
    )


def _fuse_dpairs(dw):
    t = _tile_lhsT(dw)
    return np.ascontiguousarray(np.concatenate([t[0::2], t[1::2]], axis=2))


def _pack_x(xcat):
    # [CT, D] f32 -> [P, ND*CT] bf16 with row p holding all d-blocks' row p
    CT = xcat.shape[0]
    return np.ascontiguousarray(
        xcat.T.reshape(ND, P, CT).transpose(1, 0, 2).reshape(P, ND * CT)
    ).astype(ml_dtypes.bfloat16)


def _unpack_y(ya, C):
    # [P, ND*C] bf16 -> [C, D] f32
    return (
        np.asarray(ya)
        .reshape(P, ND, C)
        .transpose(2, 1, 0)
        .reshape(C, D)
        .astype(np.float32)
    )


def _prep(inputs):
    x = np.asarray(inputs["hidden_states"], dtype=np.float32).reshape(T, D)
    rw = np.asarray(inputs["router_w"], np.float32)

    # router: top-1 expert + sigmoid(max logit) scale, computed while sharding
    logits = x @ rw
    eidx = logits.argmax(-1)
    score = 1.0 / (1.0 + np.exp(-logits.max(-1)))
    xs = x * score[:, None]

    idx = [np.nonzero(eidx == c)[0] for c in range(N_CORES)]
    maxn = max(len(i) for i in idx)
    C1 = max(16, -(-maxn // 16) * 16)
    CT = C1 + C2

    sgu_t = _fuse_gu(
        np.asarray(inputs["shared_gate_w"], np.float32),
        np.asarray(inputs["shared_up_w"], np.float32),
    )
    sdp_t = _fuse_dpairs(np.asarray(inputs["shared_down_w"], np.float32))
    gw_all = np.asarray(inputs["gate_w"], np.float32)
    uw_all = np.asarray(inputs["up_w"], np.float32)
    dw_all = np.asarray(inputs["down_w"], np.float32)

    in_maps = []
    for c in range(N_CORES):
        xcat = np.zeros((CT, D), np.float32)
        xcat[: len(idx[c])] = xs[idx[c]]
        xcat[C1:] = x[C2 * c : C2 * (c + 1)]
        in_maps.append(
            {
                "xa": _pack_x(xcat),
                "wgu": _fuse_gu(gw_all[c], uw_all[c]),
                "wdp": _fuse_dpairs(dw_all[c]),
                "sgu": sgu_t,
                "sdp": sdp_t,
            }
        )
    return in_maps, idx, C1


def run(inputs, trace=False, tmpdir=None):
    from concourse.bass_utils import run_bass_kernel_spmd

    in_maps, idx, C1 = _prep(inputs)
    CT = C1 + C2
    nc = build(C1)
    res = run_bass_kernel_spmd(
        nc, in_maps, core_ids=list(range(N_CORES)), trace=trace, tmpdir=tmpdir
    )
    out = np.zeros((T, D), np.float32)
    for c in range(N_CORES):
        ye = _unpack_y(res.results[c]["ye"], C1)
        ys = _unpack_y(res.results[c]["ys"], C2)
        out[idx[c]] += ye[: len(idx[c])]
        out[C2 * c : C2 * (c + 1)] += ys
    return out.reshape(T // 2, 2, D), res


def kernel(**inputs) -> np.ndarray:
    out, _ = run(inputs)
    return out


# revision 15
# speedup vs baseline: 1.0965x; 1.0965x over previous
"""Llama4 MoE layer on 8 Trainium2 NeuronCores — expert-parallel sparse dispatch.

Sharding strategy (the "all-to-all dispatch of top-1 routed tokens" from the
hint, done at the input-sharding step): the router is evaluated while sharding
the inputs, tokens are gathered per top-1 expert, and core c receives

  - the tokens routed to expert c (pre-scaled by sigmoid(max logit)), padded
    to C1 columns, plus
  - an even 1/8 slice of all tokens (unscaled) for the shared expert,

so each core runs ~C1+256 token-MLPs instead of the dense-masked 2048+256.
Expert outputs live on disjoint token sets and the shared slices tile the
token axis, so combining is a scatter-add — no collective needed.

Device kernel per core (identical SPMD program):
  x [P, ND*CT] bf16 -> gate/up (weight tiles stationary as lhsT, tokens
  stream) -> silu*up in f32 PSUM -> h [F, C] bf16 -> down-proj in flipped
  orientation (down tiles stationary, h streams) -> y [P, ND*CT] bf16,
  written back in one DMA at the end.
All matmuls use a full 128x128 stationary operand, so PE time ~= FLOPs/peak.
DMA-efficiency choices: gate|up fused per f-tile (16KB partition rows),
down tiles fused in pairs (8KB rows), x loaded in 4 wide-row DMAs, output
accumulated in SBUF and stored once (bf16).
"""

import sys

sys.path.insert(0, "/opt/trn_rl_repo")

import ml_dtypes
import numpy as np

import concourse.tile as tile
from concourse import bacc, mybir

T, D, F, E = 2048, 2048, 2048, 8
N_CORES = 8
P = 128
ND, NF = D // P, F // P
C2 = T // N_CORES  # shared-expert tokens per core
f32 = mybir.dt.float32
bf16 = mybir.dt.bfloat16


def build(C1):
    CT = C1 + C2
    nc = bacc.Bacc(None, target_bir_lowering=False, debug=False)
    xa = nc.declare_dram_parameter("xa", [P, ND * CT], bf16, isOutput=False)
    wgu = nc.declare_dram_parameter("wgu", [NF, P, 2 * ND * P], bf16, isOutput=False)
    wdp = nc.declare_dram_parameter(
        "wdp", [ND // 2, P, 2 * NF * P], bf16, isOutput=False
    )
    sgu = nc.declare_dram_parameter("sgu", [NF, P, 2 * ND * P], bf16, isOutput=False)
    sdp = nc.declare_dram_parameter(
        "sdp", [ND // 2, P, 2 * NF * P], bf16, isOutput=False
    )
    ye = nc.declare_dram_parameter("ye", [P, ND * C1], bf16, isOutput=True)
    ys = nc.declare_dram_parameter("ys", [P, ND * C2], bf16, isOutput=True)

    with tile.TileContext(nc) as tc:
        with (
            tc.tile_pool(name="xpool", bufs=1) as xp,
            tc.tile_pool(name="wstream", bufs=5) as wp,
            tc.tile_pool(name="hpool", bufs=2) as hp,
            tc.tile_pool(name="work", bufs=2) as sp,
            tc.tile_pool(name="psGU", bufs=2, space="PSUM") as ppG,
            tc.tile_pool(name="psD", bufs=2, space="PSUM") as ppD,
        ):
            # first expert weight tile before x so its packets interleave with
            # the x load instead of queueing behind it
            gu00 = wp.tile([P, 2 * ND * P], bf16, tag="wgu", name="gu0_0")
            nc.sync.dma_start(out=gu00[:, : ND * P], in_=wgu[0, :, : ND * P])
            xa_t = xp.tile([P, ND * CT], bf16, tag="xa", name="xa_t")
            nsplit = 4
            step = (ND // nsplit) * CT
            for s in range(nsplit):
                nc.sync.dma_start(
                    out=xa_t[:, step * s : step * (s + 1)],
                    in_=xa[:, step * s : step * (s + 1)],
                )
            nc.sync.dma_start(out=gu00[:, ND * P :], in_=wgu[0, :, ND * P :])
            xt = [xa_t[:, CT * d : CT * (d + 1)] for d in range(ND)]
            ye_t = xp.tile([P, ND * C1], bf16, tag="ye", name="ye_t")
            ys_t = xp.tile([P, ND * C2], bf16, tag="ys", name="ys_t")

            for w, (gu_p, dp_p, c0, C, y_t, y_p, CW) in enumerate(
                [(wgu, wdp, 0, C1, ye_t, ye, C1), (sgu, sdp, C1, C2, ys_t, ys, C2)]
            ):
                # token chunks of <=512 (PSUM bank width in f32)
                chunks = []
                q0 = 0
                while q0 < C:
                    qw = min(512, C - q0)
                    chunks.append((q0, qw))
                    q0 += qw
                # ---- gate/up -> h[f] [P, C] bf16 ----
                h_tiles = []
                for f in range(NF):
                    if w == 0 and f == 0:
                        gu = gu00  # preloaded before the x DMAs
                    else:
                        gu = wp.tile(
                            [P, 2 * ND * P], bf16, tag="wgu", name=f"gu{w}_{f}"
                        )
                        nc.sync.dma_start(out=gu[:], in_=gu_p[f])
                    gt = gu[:, : ND * P]
                    ut = gu[:, ND * P :]
                    h_t = hp.tile([P, C], bf16, tag=f"h{f}", name=f"h{w}_{f}")
                    for q0, qw in chunks:
                        pg = ppG.tile([P, qw], f32, space="PSUM", tag="pg", name="pg")
                        pu = ppG.tile([P, qw], f32, space="PSUM", tag="pu", name="pu")
                        for d in range(ND):
                            nc.tensor.matmul(
                                out=pg[:],
                                lhsT=gt[:, P * d : P * (d + 1)],
                                rhs=xt[d][:, c0 + q0 : c0 + q0 + qw],
                                start=(d == 0),
                                stop=(d == ND - 1),
                            )
                        for d in range(ND):
                            nc.tensor.matmul(
                                out=pu[:],
                                lhsT=ut[:, P * d : P * (d + 1)],
                                rhs=xt[d][:, c0 + q0 : c0 + q0 + qw],
                                start=(d == 0),
                                stop=(d == ND - 1),
                            )
                        sig = sp.tile([P, qw], f32, tag="sig", name="sig")
                        nc.scalar.activation(
                            sig[:], pg[:], mybir.ActivationFunctionType.Sigmoid
                        )
                        nc.vector.tensor_tensor(
                            out=sig[:], in0=sig[:], in1=pg[:], op=mybir.AluOpType.mult
                        )
                        nc.vector.tensor_tensor(
                            out=h_t[:, q0 : q0 + qw],
                            in0=sig[:],
                            in1=pu[:],
                            op=mybir.AluOpType.mult,
                        )
                    h_tiles.append(h_t)
                # ---- down-proj, flipped: down tiles stationary, h streams ----
                for j in range(ND // 2):
                    dd = wp.tile(
                        [P, 2 * NF * P], bf16, tag="wd", bufs=4, name=f"dd{w}_{j}"
                    )
                    nc.sync.dma_start(out=dd[:], in_=dp_p[j])
                    for half in range(2):
                        dblk = 2 * j + half
                        dt_ = dd[:, NF * P * half : NF * P * (half + 1)]
                        for q0, qw in chunks:
                            py = ppD.tile(
                                [P, qw], f32, space="PSUM", tag="py", name="py"
                            )
                            for f in range(NF):
                                nc.tensor.matmul(
                                    out=py[:],
                                    lhsT=dt_[:, P * f : P * (f + 1)],
                                    rhs=h_tiles[f][:, q0 : q0 + qw],
                                    start=(f == 0),
                                    stop=(f == NF - 1),
                                )
                            nc.vector.tensor_copy(
                                y_t[:, CW * dblk + q0 : CW * dblk + q0 + qw],
                                py[:],
                            )
                    # stream finished output strips out while compute continues
                    if j % 2 == 1:
                        nc.sync.dma_start(
                            out=y_p[:, CW * 2 * (j - 1) : CW * 2 * (j + 1)],
                            in_=y_t[:, CW * 2 * (j - 1) : CW * 2 * (j + 1)],
                        )
    nc.finalize()
    return nc


def _tile_lhsT(w):
    # [A, B] f32 -> [B/P, P, A] bf16 : block b, partition p(a%P), col a_blk*P+q
    A, B = w.shape
    return np.ascontiguousarray(
        w.reshape(A // P, P, B // P, P).transpose(2, 1, 0, 3).reshape(B // P, P, A)
    ).astype(ml_dtypes.bfloat16)


def _fuse_gu(g, u):
    return np.ascontiguousarray(
        np.concatenate([_tile_lhsT(g), _tile_lhsT(u)], axis=2)
    )


def _fuse_dpairs(dw):
    t = _tile_lhsT(dw)
    return np.ascontiguousarray(np.concatenate([t[0::2], t[1::2]], axis=2))


def _pack_x(xcat):
    # [CT, D] f32 -> [P, ND*CT] bf16 with row p holding all d-blocks' row p
    CT = xcat.shape[0]
    return np.ascontiguousarray(
        xcat.T.reshape(ND, P, CT).transpose(1, 0, 2).reshape(P, ND * CT)
    ).astype(ml_dtypes.bfloat16)


def _unpack_y(ya, C):
    # [P, ND*C] bf16 -> [C, D] f32
    return (
        np.asarray(ya)
        .reshape(P, ND, C)
        .transpose(2, 1, 0)
        .reshape(C, D)
        .astype(np.float32)
    )


def _prep(inputs):
    x = np.asarray(inputs["hidden_states"], dtype=np.float32).reshape(T, D)
    rw = np.asarray(inputs["router_w"], np.float32)

    # router: top-1 expert + sigmoid(max logit) scale, computed while sharding
    logits = x @ rw
    eidx = logits.argmax(-1)
    score = 1.0 / (1.0 + np.exp(-logits.max(-1)))
    xs = x * score[:, None]

    idx = [np.nonzero(eidx == c)[0] for c in range(N_CORES)]
    maxn = max(len(i) for i in idx)
    C1 = max(16, -(-maxn // 16) * 16)
    CT = C1 + C2

    sgu_t = _fuse_gu(
        np.asarray(inputs["shared_gate_w"], np.float32),
        np.asarray(inputs["shared_up_w"], np.float32),
    )
    sdp_t = _fuse_dpairs(np.asarray(inputs["shared_down_w"], np.float32))
    gw_all = np.asarray(inputs["gate_w"], np.float32)
    uw_all = np.asarray(inputs["up_w"], np.float32)
    dw_all = np.asarray(inputs["down_w"], np.float32)

    in_maps = []
    for c in range(N_CORES):
        xcat = np.zeros((CT, D), np.float32)
        xcat[: len(idx[c])] = xs[idx[c]]
        xcat[C1:] = x[C2 * c : C2 * (c + 1)]
        in_maps.append(
            {
                "xa": _pack_x(xcat),
                "wgu": _fuse_gu(gw_all[c], uw_all[c]),
                "wdp": _fuse_dpairs(dw_all[c]),
                "sgu": sgu_t,
                "sdp": sdp_t,
            }
        )
    return in_maps, idx, C1


def run(inputs, trace=False, tmpdir=None):
    from concourse.bass_utils import run_bass_kernel_spmd

    in_maps, idx, C1 = _prep(inputs)
    CT = C1 + C2
    nc = build(C1)
    res = run_bass_kernel_spmd(
        nc, in_maps, core_ids=list(range(N_CORES)), trace=trace, tmpdir=tmpdir
    )
    out = np.zeros((T, D), np.float32)
    for c in range(N_CORES):
        ye = _unpack_y(res.results[c]["ye"], C1)
        ys = _unpack_y(res.results[c]["ys"], C2)
        out[idx[c]] += ye[: len(idx[c])]
        out[C2 * c : C2 * (c + 1)] += ys
    return out.reshape(T // 2, 2, D), res


def kernel(**inputs) -> np.ndarray:
    out, _ = run(inputs)
    return out


# revision 17
# speedup vs baseline: 1.1057x; 1.0084x over previous
"""Llama4 MoE layer on 8 Trainium2 NeuronCores — expert-parallel sparse dispatch.

Sharding strategy (the "all-to-all dispatch of top-1 routed tokens" from the
hint, done at the input-sharding step): the router is evaluated while sharding
the inputs, tokens are gathered per top-1 expert, and core c receives

  - the tokens routed to expert c (pre-scaled by sigmoid(max logit)), padded
    to C1 columns, plus
  - an even 1/8 slice of all tokens (unscaled) for the shared expert,

so each core runs ~C1+256 token-MLPs instead of the dense-masked 2048+256.
Expert outputs live on disjoint token sets and the shared slices tile the
token axis, so combining is a scatter-add — no collective needed.

Device kernel per core (identical SPMD program):
  x [P, ND*CT] bf16 -> gate/up (weight tiles stationary as lhsT, tokens
  stream) -> silu*up in f32 PSUM -> h [F, C] bf16 -> down-proj in flipped
  orientation (down tiles stationary, h streams) -> y [P, ND*CT] bf16,
  written back in one DMA at the end.
All matmuls use a full 128x128 stationary operand, so PE time ~= FLOPs/peak.
DMA-efficiency choices: gate|up fused per f-tile (16KB partition rows),
down tiles fused in pairs (8KB rows), x loaded in 4 wide-row DMAs, output
accumulated in SBUF and stored once (bf16).
"""

import sys

sys.path.insert(0, "/opt/trn_rl_repo")

import ml_dtypes
import numpy as np

import concourse.tile as tile
from concourse import bacc, mybir

T, D, F, E = 2048, 2048, 2048, 8
N_CORES = 8
P = 128
ND, NF = D // P, F // P
C2 = T // N_CORES  # shared-expert tokens per core
f32 = mybir.dt.float32
bf16 = mybir.dt.bfloat16


def build(C1):
    CT = C1 + C2
    nc = bacc.Bacc(None, target_bir_lowering=False, debug=False)
    xa = nc.declare_dram_parameter("xa", [P, ND * CT], bf16, isOutput=False)
    wgu = nc.declare_dram_parameter("wgu", [NF, P, 2 * ND * P], bf16, isOutput=False)
    wdp = nc.declare_dram_parameter(
        "wdp", [ND // 2, P, 2 * NF * P], bf16, isOutput=False
    )
    sgu = nc.declare_dram_parameter("sgu", [NF, P, 2 * ND * P], bf16, isOutput=False)
    sdp = nc.declare_dram_parameter(
        "sdp", [ND // 2, P, 2 * NF * P], bf16, isOutput=False
    )
    ye = nc.declare_dram_parameter("ye", [P, ND * C1], bf16, isOutput=True)
    ys = nc.declare_dram_parameter("ys", [P, ND * C2], bf16, isOutput=True)

    with tile.TileContext(nc) as tc:
        with (
            tc.tile_pool(name="xpool", bufs=1) as xp,
            tc.tile_pool(name="wstream", bufs=5) as wp,
            tc.tile_pool(name="hpool", bufs=2) as hp,
            tc.tile_pool(name="work", bufs=2) as sp,
            tc.tile_pool(name="psGU", bufs=2, space="PSUM") as ppG,
            tc.tile_pool(name="psD", bufs=2, space="PSUM") as ppD,
        ):
            # first expert weight tile before x so its packets interleave with
            # the x load instead of queueing behind it
            gu00 = wp.tile([P, 2 * ND * P], bf16, tag="wgu", name="gu0_0")
            nc.sync.dma_start(out=gu00[:, : ND * P], in_=wgu[0, :, : ND * P])
            xa_t = xp.tile([P, ND * CT], bf16, tag="xa", name="xa_t")
            nsplit = 4
            step = (ND // nsplit) * CT
            for s in range(nsplit):
                nc.sync.dma_start(
                    out=xa_t[:, step * s : step * (s + 1)],
                    in_=xa[:, step * s : step * (s + 1)],
                )
            nc.sync.dma_start(out=gu00[:, ND * P :], in_=wgu[0, :, ND * P :])
            xt = [xa_t[:, CT * d : CT * (d + 1)] for d in range(ND)]

            # HAM pre-warm: ~5us of dummy PE activity while the x/weight DMAs
            # land, so the clock gate is at 8/8 when real matmuls start
            warm = xp.tile([P, P], bf16, tag="warm", name="warm")
            nc.vector.memset(warm[:], 0.0)
            wps = ppG.tile([P, 64], f32, space="PSUM", tag="warm", bufs=1, name="wps")
            for _ in range(22):
                nc.tensor.matmul(
                    out=wps[:], lhsT=warm[:], rhs=warm[:, :64], start=True, stop=True
                )
            ye_t = xp.tile([P, ND * C1], bf16, tag="ye", name="ye_t")
            ys_t = xp.tile([P, ND * C2], bf16, tag="ys", name="ys_t")

            for w, (gu_p, dp_p, c0, C, y_t, y_p, CW) in enumerate(
                [(wgu, wdp, 0, C1, ye_t, ye, C1), (sgu, sdp, C1, C2, ys_t, ys, C2)]
            ):
                # token chunks of <=512 (PSUM bank width in f32)
                chunks = []
                q0 = 0
                while q0 < C:
                    qw = min(512, C - q0)
                    chunks.append((q0, qw))
                    q0 += qw
                # ---- gate/up -> h[f] [P, C] bf16 ----
                h_tiles = []
                for f in range(NF):
                    if w == 0 and f == 0:
                        gu = gu00  # preloaded before the x DMAs
                    else:
                        gu = wp.tile(
                            [P, 2 * ND * P], bf16, tag="wgu", name=f"gu{w}_{f}"
                        )
                        nc.sync.dma_start(out=gu[:], in_=gu_p[f])
                    gt = gu[:, : ND * P]
                    ut = gu[:, ND * P :]
                    h_t = hp.tile([P, C], bf16, tag=f"h{f}", name=f"h{w}_{f}")
                    for q0, qw in chunks:
                        pg = ppG.tile([P, qw], f32, space="PSUM", tag="pg", name="pg")
                        pu = ppG.tile([P, qw], f32, space="PSUM", tag="pu", name="pu")
                        for d in range(ND):
                            nc.tensor.matmul(
                                out=pg[:],
                                lhsT=gt[:, P * d : P * (d + 1)],
                                rhs=xt[d][:, c0 + q0 : c0 + q0 + qw],
                                start=(d == 0),
                                stop=(d == ND - 1),
                            )
                        for d in range(ND):
                            nc.tensor.matmul(
                                out=pu[:],
                                lhsT=ut[:, P * d : P * (d + 1)],
                                rhs=xt[d][:, c0 + q0 : c0 + q0 + qw],
                                start=(d == 0),
                                stop=(d == ND - 1),
                            )
                        sig = sp.tile([P, qw], f32, tag="sig", name="sig")
                        nc.scalar.activation(
                            sig[:], pg[:], mybir.ActivationFunctionType.Sigmoid
                        )
                        nc.vector.tensor_tensor(
                            out=sig[:], in0=sig[:], in1=pg[:], op=mybir.AluOpType.mult
                        )
                        nc.vector.tensor_tensor(
                            out=h_t[:, q0 : q0 + qw],
                            in0=sig[:],
                            in1=pu[:],
                            op=mybir.AluOpType.mult,
                        )
                    h_tiles.append(h_t)
                # ---- down-proj, flipped: down tiles stationary, h streams ----
                for j in range(ND // 2):
                    dd = wp.tile(
                        [P, 2 * NF * P], bf16, tag="wd", bufs=4, name=f"dd{w}_{j}"
                    )
                    nc.sync.dma_start(out=dd[:], in_=dp_p[j])
                    for half in range(2):
                        dblk = 2 * j + half
                        dt_ = dd[:, NF * P * half : NF * P * (half + 1)]
                        for q0, qw in chunks:
                            py = ppD.tile(
                                [P, qw], f32, space="PSUM", tag="py", name="py"
                            )
                            for f in range(NF):
                                nc.tensor.matmul(
                                    out=py[:],
                                    lhsT=dt_[:, P * f : P * (f + 1)],
                                    rhs=h_tiles[f][:, q0 : q0 + qw],
                                    start=(f == 0),
                                    stop=(f == NF - 1),
                                )
                            nc.vector.tensor_copy(
                                y_t[:, CW * dblk + q0 : CW * dblk + q0 + qw],
                                py[:],
                            )
                    # stream finished output strips out while compute continues
                    nc.sync.dma_start(
                        out=y_p[:, CW * 2 * j : CW * 2 * (j + 1)],
                        in_=y_t[:, CW * 2 * j : CW * 2 * (j + 1)],
                    )
    nc.finalize()
    return nc


def _tile_lhsT(w):
    # [A, B] f32 -> [B/P, P, A] bf16 : block b, partition p(a%P), col a_blk*P+q
    A, B = w.shape
    return np.ascontiguousarray(
        w.reshape(A // P, P, B // P, P).transpose(2, 1, 0, 3).reshape(B // P, P, A)
    ).astype(ml_dtypes.bfloat16)


def _fuse_gu(g, u):
    return np.ascontiguousarray(
        np.concatenate([_tile_lhsT(g), _tile_lhsT(u)], axis=2)
    )


def _fuse_dpairs(dw):
    t = _tile_lhsT(dw)
    return np.ascontiguousarray(np.concatenate([t[0::2], t[1::2]], axis=2))


def _pack_x(xcat):
    # [CT, D] f32 -> [P, ND*CT] bf16 with row p holding all d-blocks' row p
    CT = xcat.shape[0]
    return np.ascontiguousarray(
        xcat.T.reshape(ND, P, CT).transpose(1, 0, 2).reshape(P, ND * CT)
    ).astype(ml_dtypes.bfloat16)


def _unpack_y(ya, C):
    # [P, ND*C] bf16 -> [C, D] f32
    return (
        np.asarray(ya)
        .reshape(P, ND, C)
        .transpose(2, 1, 0)
        .reshape(C, D)
        .astype(np.float32)
    )


def _prep(inputs):
    x = np.asarray(inputs["hidden_states"], dtype=np.float32).reshape(T, D)
    rw = np.asarray(inputs["router_w"], np.float32)

    # router: top-1 expert + sigmoid(max logit) scale, computed while sharding
    logits = x @ rw
    eidx = logits.argmax(-1)
    score = 1.0 / (1.0 + np.exp(-logits.max(-1)))
    xs = x * score[:, None]

    idx = [np.nonzero(eidx == c)[0] for c in range(N_CORES)]
    maxn = max(len(i) for i in idx)
    C1 = max(16, -(-maxn // 16) * 16)
    CT = C1 + C2

    sgu_t = _fuse_gu(
        np.asarray(inputs["shared_gate_w"], np.float32),
        np.asarray(inputs["shared_up_w"], np.float32),
    )
    sdp_t = _fuse_dpairs(np.asarray(inputs["shared_down_w"], np.float32))
    gw_all = np.asarray(inputs["gate_w"], np.float32)
    uw_all = np.asarray(inputs["up_w"], np.float32)
    dw_all = np.asarray(inputs["down_w"], np.float32)

    in_maps = []
    for c in range(N_CORES):
        xcat = np.zeros((CT, D), np.float32)
        xcat[: len(idx[c])] = xs[idx[c]]
        xcat[C1:] = x[C2 * c : C2 * (c + 1)]
        in_maps.append(
            {
                "xa": _pack_x(xcat),
                "wgu": _fuse_gu(gw_all[c], uw_all[c]),
                "wdp": _fuse_dpairs(dw_all[c]),
                "sgu": sgu_t,
                "sdp": sdp_t,
            }
        )
    return in_maps, idx, C1


def run(inputs, trace=False, tmpdir=None):
    from concourse.bass_utils import run_bass_kernel_spmd

    in_maps, idx, C1 = _prep(inputs)
    CT = C1 + C2
    nc = build(C1)
    res = run_bass_kernel_spmd(
        nc, in_maps, core_ids=list(range(N_CORES)), trace=trace, tmpdir=tmpdir
    )
    out = np.zeros((T, D), np.float32)
    for c in range(N_CORES):
        ye = _unpack_y(res.results[c]["ye"], C1)
        ys = _unpack_y(res.results[c]["ys"], C2)
        out[idx[c]] += ye[: len(idx[c])]
        out[C2 * c : C2 * (c + 1)] += ys
    return out.reshape(T // 2, 2, D), res


def kernel(**inputs) -> np.ndarray:
    out, _ = run(inputs)
    return out
